# revision 1
# baseline (speedup 1.0000x reference)
"""Bidirectional GRU decoder on 8 Trainium2 NeuronCores.

Strategy (pure data parallelism over batch, per the sharding hint):
  - batch 8192 -> 1024 per core; inside a core, 4 batch groups of 256.
  - Per time step, each gate (r, z, nh, ni) is one matmul with a
    block-diagonal lhsT covering all 4 groups at once, so downstream
    elementwise/activation ops run on 96 partitions.
  - rhs row layout: 0:96 h (4 groups x 24), 96 ones (bias row),
    97:105 x (4 groups x 2). Fwd and bwd directions run in the same
    loop (bwd consumes time-reversed x), packed into separate column
    halves of shared psum/sbuf tiles.
  - Output projection w_out . h_t rides as a small accumulating matmul
    (M = 32 = 8 time-slots x 4 groups) on the same rhs stream, with a
    sliding-window block-diagonal lhsT; evacuated every 8 steps.
  - All constants (gate lhsTs, proj windows, initial rhs images) live
    in ONE SBUF tile loaded by ONE DMA so no matmul ever needs more
    than one sync wait (HW limit: one wait per LDWEIGHTS).
  - Host (numpy) does all weight/layout preprocessing and the final
    unscramble + b_out add; the device kernel does the serial scan.
"""
import numpy as np

H = 24
D = 2
T = 262
K_INFO = 256
B = 8192
N_CORES = 8
B_C = B // N_CORES          # 1024 batch per core
NG = 4                      # batch groups per core
G = B_C // NG               # 256 batch per group
N = G                       # matmul free dim per direction
W = 8                       # proj window steps
KROWS = NG * H + 1 + NG * D  # 105: h 0:96, ones 96, x 97:105
PWCOLS = 8 * W + 4 * (W - 1)  # 92: sliding dual-dir proj window buffer

# wblock free-dim element offsets (fp32)
WOFF_LHST = [[g_i * 96 + d_i * 4 * 96 for g_i in range(4)] for d_i in range(2)]
WOFF_PW = 8 * 96
WOFF_RHS = [8 * 96 + PWCOLS, 8 * 96 + PWCOLS + 2 * N]
WBLOCK_F = 8 * 96 + PWCOLS + 4 * N


def _n_win(t_steps):
    return (t_steps + 1 + W - 1) // W


# ---------------------------------------------------------------- host prep

def _build_gate_lhsts(w_ih, w_hh, b_ih, b_hh):
    """Returns [4, KROWS, 96] for gates r, z, nh, ni (unused rows zero)."""
    out = np.zeros((4, KROWS, 96), np.float32)
    for gi, gate in enumerate([0, 1]):  # r, z: h + x + both biases
        s = gate * H
        for g in range(NG):
            out[gi, H * g:H * g + H, H * g:H * g + H] = w_hh[s:s + H].T
            out[gi, 97 + D * g:97 + D * g + D, H * g:H * g + H] = \
                w_ih[s:s + H].T
            out[gi, 96, H * g:H * g + H] = b_ih[s:s + H] + b_hh[s:s + H]
    s = 2 * H
    for g in range(NG):  # nh: h + b_hh ; ni: x + b_ih
        out[2, H * g:H * g + H, H * g:H * g + H] = w_hh[s:s + H].T
        out[2, 96, H * g:H * g + H] = b_hh[s:s + H]
        out[3, 97 + D * g:97 + D * g + D, H * g:H * g + H] = w_ih[s:s + H].T
        out[3, 96, H * g:H * g + H] = b_ih[s:s + H]
    return out


def _build_proj_win(w_out):
    """Sliding-window buffer [96, PWCOLS]; window for slot s is
    buf[:, 4*(W-1)-4s : +8W], placing the fwd blockdiag at local cols
    4s:4s+4 and the bwd blockdiag at 4W+4s:4W+4s+4 (one M=64 matmul
    covers both directions; cross-direction output quadrants are unused)."""
    buf = np.zeros((96, PWCOLS), np.float32)
    for g in range(NG):
        buf[H * g:H * g + H, 4 * (W - 1) + g] = w_out[0, :H]
        buf[H * g:H * g + H, 4 * (W - 1) + 4 * W + g] = w_out[0, H:]
    return buf


def _pack_xs(x_core, t_steps):
    """x_core [B_C, T, D] -> xs [t_steps, NG*D, 2N] (f|b halves)."""
    xg = x_core.reshape(NG, G, x_core.shape[1], D)
    xs = np.zeros((t_steps, NG * D, 2 * N), np.float32)
    for t in range(t_steps):
        for g in range(NG):
            xs[t, D * g:D * g + D, 0:N] = xg[g, :, t, :].T
            xs[t, D * g:D * g + D, N:2 * N] = xg[g, :, t_steps - 1 - t, :].T
    return xs


def _build_wblock(weights, xs0):
    """One [KROWS, WBLOCK_F] constant block: 8 gate lhsTs, 2 proj windows,
    2 initial rhs images (h=0, ones row, x-rows = xs[0] for rhs0)."""
    (w_ih_f, w_hh_f, b_ih_f, b_hh_f, w_ih_b, w_hh_b, b_ih_b, b_hh_b,
     w_out) = weights
    wb = np.zeros((KROWS, WBLOCK_F), np.float32)
    for d_i, args in enumerate([(w_ih_f, w_hh_f, b_ih_f, b_hh_f),
                                (w_ih_b, w_hh_b, b_ih_b, b_hh_b)]):
        lh = _build_gate_lhsts(*args)
        for g_i in range(4):
            wb[:, WOFF_LHST[d_i][g_i]:WOFF_LHST[d_i][g_i] + 96] = lh[g_i]
    wb[0:96, WOFF_PW:WOFF_PW + PWCOLS] = _build_proj_win(w_out)
    for i in range(2):
        wb[96, WOFF_RHS[i]:WOFF_RHS[i] + 2 * N] = 1.0
    wb[97:KROWS, WOFF_RHS[0]:WOFF_RHS[0] + 2 * N] = xs0
    return wb


# ---------------------------------------------------------------- bass build

def build_nc(t_steps=T):
    import concourse.bass as bass
    import concourse.tile as tile
    from concourse import mybir
    from contextlib import ExitStack

    f32 = mybir.dt.float32
    nwin = _n_win(t_steps)

    nc = bass.Bass()
    xs_d = nc.dram_tensor("xs", [t_steps, NG * D, 2 * N], f32,
                          kind="ExternalInput")
    wb_d = nc.dram_tensor("wblock", [KROWS, WBLOCK_F], f32,
                          kind="ExternalInput")
    out_d = nc.dram_tensor("proj_out", [nwin, 8 * W, 2 * N], f32,
                           kind="ExternalOutput")

    with tile.TileContext(nc) as tc, ExitStack() as ctx:
        wpool = ctx.enter_context(tc.tile_pool(name="weights", bufs=1))
        spool = ctx.enter_context(tc.tile_pool(name="work", bufs=3))
        ps_rz_pool = ctx.enter_context(
            tc.tile_pool(name="ps_rz", bufs=2, space="PSUM"))
        ps_n_pool = ctx.enter_context(
            tc.tile_pool(name="ps_n", bufs=1, space="PSUM"))
        ps_p_pool = ctx.enter_context(
            tc.tile_pool(name="ps_p", bufs=2, space="PSUM"))

        wb = wpool.tile([KROWS, WBLOCK_F], f32, tag="wb", name="wb")
        nc.sync.dma_start(out=wb, in_=wb_d[:])
        krows_by_gate = [KROWS, KROWS, 97, KROWS]
        lw = {}
        for d_i in range(2):
            for g_i in range(4):
                off = WOFF_LHST[d_i][g_i]
                lw[(d_i, g_i)] = wb[0:krows_by_gate[g_i], off:off + 96]
        pw = wb[0:96, WOFF_PW:WOFF_PW + PWCOLS]
        rhs = [wb[:, WOFF_RHS[i]:WOFF_RHS[i] + 2 * N] for i in range(2)]
        # persistent evacuation buffer: one region per window, never reused,
        # so the evac copy never carries a WAR wait against an out-DMA
        evbuf = wpool.tile([8 * W, nwin * 2 * N], f32, tag="evb", name="evb")

        def q(ap, start):  # quarter-strided view [96, 2, N]
            return ap.rearrange("p (q c) -> p q c", q=4)[:, start::2, :]

        def h2(ap):  # [96, 2N] -> [96, 2, N]
            return ap.rearrange("p (q c) -> p q c", q=2)

        jb = spool.tile([96, 1], f32, tag="jb", name="jb")
        nc.vector.tensor_copy(jb, rhs[0][0:96, 0:1])
        proj_ps = None
        for t in range(t_steps + 1):
            cur = rhs[t % 2]
            nxt = rhs[(t + 1) % 2]
            s_slot = t % W
            last = (t == t_steps)
            if s_slot == 0:
                proj_ps = ps_p_pool.tile([8 * W, 2 * N], f32, tag="pp",
                                         name="pp")
            # --- PE, ordered so each matmul carries at most one sync wait:
            # proj+nh touch only h rows (DVE wait), ni touches x rows
            # (DMA wait), r carries the psum-WAR (ACT wait), z rides free.
            win = pw[:, 4 * (W - 1) - 4 * s_slot:
                     4 * (W - 1) - 4 * s_slot + 8 * W]
            nc.tensor.matmul(
                proj_ps, win, cur[0:96, :],
                start=(s_slot == 0), stop=(s_slot == W - 1 or last))
            if not last:
                ps_rz = ps_rz_pool.tile([96, 4 * N], f32, tag="rz", name="rz")
                ps_n = ps_n_pool.tile([96, 4 * N], f32, tag="n", name="n")
                for d_i in range(2):
                    nc.tensor.matmul(
                        ps_n[:, (2 * d_i) * N:(2 * d_i + 1) * N],
                        lw[(d_i, 2)], cur[0:97, d_i * N:(d_i + 1) * N],
                        start=True, stop=True)
                for d_i in range(2):
                    nc.tensor.matmul(
                        ps_n[:, (2 * d_i + 1) * N:(2 * d_i + 2) * N],
                        lw[(d_i, 3)], cur[:, d_i * N:(d_i + 1) * N],
                        start=True, stop=True)
                for d_i in range(2):
                    r_ap = cur[:, d_i * N:(d_i + 1) * N]
                    nc.tensor.matmul(
                        ps_rz[:, (2 * d_i) * N:(2 * d_i + 1) * N],
                        lw[(d_i, 0)], r_ap, start=True, stop=True)
                    nc.tensor.matmul(
                        ps_rz[:, (2 * d_i + 1) * N:(2 * d_i + 2) * N],
                        lw[(d_i, 1)], r_ap, start=True, stop=True)
            if s_slot == W - 1 or last:
                wdx = t // W
                ev = evbuf[:, wdx * 2 * N:(wdx + 1) * 2 * N]
                nc.vector.tensor_copy(ev, proj_ps)
            if last:
                break

            rz_sb = spool.tile([96, 4 * N], f32, tag="rz_sb", name="rz_sb")
            # split sigmoid: r first (on the critical path into m), z after
            # (only needed by e, which waits for tanh anyway)
            nc.scalar.activation(q(rz_sb, 0), q(ps_rz, 0),
                                 mybir.ActivationFunctionType.Sigmoid)
            nc.scalar.activation(q(rz_sb, 1), q(ps_rz, 1),
                                 mybir.ActivationFunctionType.Sigmoid)
            c_t = spool.tile([96, 2 * N], f32, tag="c", name="c")
            nc.scalar.activation(h2(c_t), q(ps_rz, 1),
                                 mybir.ActivationFunctionType.Sigmoid,
                                 scale=-1.0)
            hp = spool.tile([96, 4], f32, tag="hp", name="hp")
            nc.vector.tensor_copy(
                out=hp[:].rearrange("p (q c) -> p q c", q=4),
                in_=ps_n.rearrange("p (q c) -> p q c", q=4)[:, :, 0:1])
            m_t = spool.tile([96, 2 * N], f32, tag="m", name="m")
            nc.vector.tensor_tensor(out=h2(m_t), in0=q(rz_sb, 0),
                                    in1=q(ps_n, 0), op=mybir.AluOpType.mult)
            s_t = spool.tile([96, 2 * N], f32, tag="s", name="s")
            nc.vector.tensor_tensor(out=h2(s_t), in0=h2(m_t),
                                    in1=q(ps_n, 1), op=mybir.AluOpType.add)
            n_t = spool.tile([96, 2 * N], f32, tag="nt", name="nt")
            nc.scalar.activation(n_t, s_t, mybir.ActivationFunctionType.Tanh)
            # h' = z*h + (1-z)*n with z*h computed pre-tanh (off the chain)
            u_t = spool.tile([96, 2 * N], f32, tag="u", name="u")
            nc.vector.tensor_tensor(out=h2(u_t), in0=q(rz_sb, 1),
                                    in1=h2(cur[0:96, :]),
                                    op=mybir.AluOpType.mult)
            v_t = spool.tile([96, 2 * N], f32, tag="v", name="v")
            nc.vector.tensor_mul(v_t, n_t, c_t)
            nc.vector.tensor_add(nxt[0:96, :], u_t, v_t)
            if t + 1 < t_steps:
                nc.sync.dma_start(out=nxt[97:KROWS, :], in_=xs_d[t + 1])
        # single final output DMA: the kernel-tail drain then only needs
        # this one DMA's completion (everything else is transitively done)
        nc.sync.dma_start(out=out_d[:].rearrange("w p c -> p w c"),
                          in_=evbuf[:].rearrange("p (w c) -> p w c", w=nwin))

    _strip_same_engine_waits(nc)
    return nc


def _strip_same_engine_waits(nc):
    import concourse.mybir as mybir
    import concourse.bass as bass  # noqa
    eng_prefix = {
        mybir.EngineType.DVE: "DVE",
        mybir.EngineType.Activation: "Activation",
        mybir.EngineType.PE: "PE",
        mybir.EngineType.SP: "SP",
        mybir.EngineType.Pool: "Pool",
    }
    for blk in nc.m.functions[0].blocks:
        for inst in blk.instructions:
            si = getattr(inst, "sync_info", None)
            if not si or not si.on_wait or len(si.on_wait) < 2:
                continue
            if type(inst).__name__ == "InstDMACopy":
                continue
            pfx = eng_prefix.get(getattr(inst, "engine", None))
            if pfx is None:
                continue
            kept = [w for w in si.on_wait if not w.ant_name.startswith(pfx)]
            if kept and len(kept) < len(si.on_wait):
                si.on_wait = kept
    # xs-stream DMAs: the cross-queue WAW wait (vs the same rows' previous
    # DMA) is transitively covered by the PE wait (the matmuls that read
    # that previous DMA's data); the DMA ISA slot fits only one wait.
    for blk in nc.m.functions[0].blocks:
        for inst in blk.instructions:
            si = getattr(inst, "sync_info", None)
            if not si or not si.on_wait or len(si.on_wait) < 2:
                continue
            if type(inst).__name__ != "InstDMACopy":
                continue
            pe = [w for w in si.on_wait if w.ant_name.startswith("PE")]
            dma = [w for w in si.on_wait if w.ant_name.startswith("DMAHW")]
            if pe and dma and len(pe) + len(dma) == len(si.on_wait):
                si.on_wait = pe
    # tail drain: the final output DMA transitively dominates all other
    # work, so the multi-wait kernel-tail drain only needs that DMA's
    # completion semaphore (the ISA drain slot fits one wait)
    blocks = list(nc.m.functions[0].blocks)
    final_sem = None
    for blk in blocks:
        for inst in blk.instructions:
            if type(inst).__name__ == "InstDMACopy":
                si = getattr(inst, "sync_info", None)
                if si and si.on_update:
                    for u in si.on_update:
                        if u.ant_name.startswith("DMAHW"):
                            final_sem = u.ant_name
    for blk in blocks:
        for inst in blk.instructions:
            si = getattr(inst, "sync_info", None)
            if not si or not si.on_wait or len(si.on_wait) < 2:
                continue
            if type(inst).__name__ != "InstDrain":
                continue
            keep = [w for w in si.on_wait if w.ant_name == final_sem]
            si.on_wait = keep if keep else list(si.on_wait)[:1]


# ---------------------------------------------------------------- run + glue

def prep_inputs(x, w_ih_f, w_hh_f, b_ih_f, b_hh_f, w_ih_b, w_hh_b, b_ih_b,
                b_hh_b, w_out, b_out, t_steps=T):
    weights = (w_ih_f, w_hh_f, b_ih_f, b_hh_f, w_ih_b, w_hh_b, b_ih_b,
               b_hh_b, w_out)
    in_maps = []
    for c in range(N_CORES):
        xs = _pack_xs(x[c * B_C:(c + 1) * B_C, :t_steps], t_steps)
        wb = _build_wblock(weights, xs[0])
        in_maps.append({"xs": xs, "wblock": wb})
    return in_maps


def unshard(results, b_out, t_steps=T):
    """results: list of dicts with proj_out [nwin, 4W, 2N] per core."""
    nwin = _n_win(t_steps)
    k_info = min(K_INFO, t_steps)
    logits = np.zeros((B, k_info), np.float32)
    for c in range(N_CORES):
        po = results[c]["proj_out"]
        acc = np.zeros((B_C, k_info), np.float32)
        for wdx in range(nwin):
            for s_slot in range(min(W, t_steps + 1 - wdx * W)):
                t_step = wdx * W + s_slot
                for d_i in range(2):
                    time = t_step - 1 if d_i == 0 else t_steps - t_step
                    if not (0 <= time < k_info):
                        continue
                    row0 = 4 * s_slot + d_i * 4 * W
                    blk = po[wdx, row0:row0 + 4, d_i * N:(d_i + 1) * N]
                    acc[:, time] += blk.reshape(-1)
        logits[c * B_C:(c + 1) * B_C] = acc
    return logits + b_out[0]


def run(inputs, t_steps=T, trace=False):
    from concourse.bass_utils import run_bass_kernel_spmd
    nc = build_nc(t_steps)
    in_maps = prep_inputs(t_steps=t_steps, **inputs)
    res = run_bass_kernel_spmd(nc, in_maps, list(range(N_CORES)),
                               trace=trace)
    results = [{k: np.asarray(v) for k, v in r.items()} for r in res.results]
    out = unshard(results, inputs["b_out"], t_steps)
    return out, res


def kernel(**inputs):
    inputs = {k: np.asarray(v) for k, v in inputs.items()}
    out, _ = run(inputs)
    return out

